# revision 54
# baseline (speedup 1.0000x reference)
"""Trainium2 Bass kernel for nn_AllSparkModule (retrieval_knn).

Sharding: pure data-parallel — one batch sample per NeuronCore (8 samples,
8 cores). Cores 0-3 run the labeled (cross-attention vs kv_queue) branch,
cores 4-7 the unlabeled (channel self-attention) branch, selected at runtime
with a partition_id branch inside one SPMD program. No collectives.

Per-core pipeline (x = sample [256, 16384], channel chunks of 128):
  A: emb = gelu(w_in @ x), fp16 SBUF-resident; per-channel inorm stats
  B: norm = emb*s1+b1 (GpSimd); Q^T tiles via transposed GEMM; scores
     accumulate in PSUM over the N axis (labeled: vs kv^T in bf16;
     unlabeled: vs K^T in f32r); inorm+softmax; attn transposed via PE
  C: ca = attn @ {kv | V}; z = wo @ ca + emb (fp16 SBUF-resident) + stats
  D: out = gelu(w_out @ (z*s2+b2))

GEMMs run in float32r (full PE rate at free-dim >= 256, ~1.6e-4 accuracy);
the labeled attention uses bf16 kv (its contribution is diluted ~100x by the
residual); map_in runs in fp16. Residents emb/z are fp16 (~5e-4 total).
"""
import os
import numpy as np
import ml_dtypes

import concourse.bass as bass
import concourse.mybir as mybir
import concourse.tile as tile
from concourse.bass_utils import run_bass_kernel_spmd
from concourse.masks import make_identity
from concourse.vector_clock import ScopedClock

F32 = mybir.dt.float32
F32R = mybir.dt.float32r
BF16 = mybir.dt.bfloat16
F16 = mybir.dt.float16
AF = mybir.ActivationFunctionType
ALU = mybir.AluOpType
AX = mybir.AxisListType

P = 128          # partitions
C = 256          # channels
NCH = 2          # channel chunks
N = 16384        # feature length
TILE = 512       # free-dim tile
NSUB = TILE // P  # 128-col subtiles per tile (for transposed gemms)
NT_FULL = N // TILE
J_L = 512        # labeled keys (nc*ec)
J_U = 256        # unlabeled keys (ec)
SCALE = float(N) ** -0.5
EPS = 1e-5

BUFS_IO = int(os.environ.get("KERNEL_BUFS_IO", "3"))

# ---------------------------------------------------------------------------
# Workarounds for the pinned walrus: max ONE sync-wait per instruction.


class _TC(tile.TileContext):
    def _drain_and_barrier(self, tick_clock, wait_clock):
        drain_inst = self.nc.sync.drain()
        wait_clock.add_sem_waits(
            drain_inst.ins, ScopedClock({None: tick_clock.global_clock})
        )
        si = drain_inst.ins.sync_info
        if si is not None and si.on_wait and len(si.on_wait) > 1:
            waits = list(si.on_wait)
            drain_inst.ins.sync_info = mybir.SyncInfo(
                on_wait=waits[:1], on_update=list(si.on_update))
            for w in waits[1:]:
                d2 = self.nc.sync.drain()
                d2.ins.sync_info = mybir.SyncInfo(on_wait=[w], on_update=[])
        self.nc.all_engine_barrier()
        assert self.sems is not None
        popped = self.nc._tile_sem_poison_stack.pop()
        assert popped is self._sem_poison
        self.nc.clear_and_free_semaphores(list(self.sems.allocated().values()))
        self.nc.all_engine_barrier()


def _split_sync_waits(nc, max_waits: int = 1):
    """Rewrite sync_info for the pinned walrus: at most one wait per
    instruction, and no instruction (other than barrier EventSemaphores)
    that both waits on and updates the same semaphore. Extra/conflicting
    waits are hoisted onto NOPs inserted just before, on the same engine
    stream, which preserves ordering."""
    ctr = 0
    for fn in nc.m.functions:
        for bb in fn.blocks:
            insts = bb.instructions
            new = []
            changed = False
            for inst in insts:
                si = getattr(inst, "sync_info", None)
                waits = list(si.on_wait) if si is not None and si.on_wait else []
                upd = list(si.on_update) if si is not None and si.on_update else []
                conflict = False
                iname = type(inst).__name__
                if waits and upd and iname != "InstEventSemaphore":
                    upd_ids = {u.id for u in upd}
                    conflict = any(w.id in upd_ids for w in waits)
                    if not conflict and iname != "InstDMACopy":
                        imm_upd = any(
                            str(getattr(u, "update_mode", "")).endswith("imm")
                            for u in upd)
                        conflict = (imm_upd
                                    or inst.engine == mybir.EngineType.Pool)
                if len(waits) > max_waits or conflict:
                    keep = [] if conflict else waits[-max_waits:]
                    extras = waits if conflict else waits[:-max_waits]
                    for s in range(0, len(extras), max_waits):
                        chunk = extras[s:s + max_waits]
                        nop = mybir.InstNoOp(
                            name=f"waitsplit_{ctr}", ins=[], outs=[])
                        ctr += 1
                        nop.engine = inst.engine
                        nop.sync_info = mybir.SyncInfo(
                            on_wait=list(chunk), on_update=[])
                        new.append(nop)
                    inst.sync_info = mybir.SyncInfo(
                        on_wait=list(keep), on_update=list(upd))
                    changed = True
                new.append(inst)
            if changed:
                bb.instructions = new
    return ctr


# ---------------------------------------------------------------------------


def _build(nt: int, branch: str | None = None):
    """Build the SPMD program processing the first nt 512-column tiles."""
    nc = bass.Bass()

    x_in = nc.declare_dram_parameter("x", [C, N], F16, isOutput=False)
    kvt_in = nc.declare_dram_parameter("kvt", [N, J_L], BF16, isOutput=False)
    kv_in = nc.declare_dram_parameter("kv", [J_L, N], BF16, isOutput=False)
    w_inT = nc.declare_dram_parameter("w_inT", [C, C], F16, isOutput=False)
    wqT = nc.declare_dram_parameter("wqT", [C, C], F32R, isOutput=False)
    wkT = nc.declare_dram_parameter("wkT", [C, C], F32R, isOutput=False)
    wvT = nc.declare_dram_parameter("wvT", [C, C], F32R, isOutput=False)
    woT = nc.declare_dram_parameter("woT", [C, C], F32R, isOutput=False)
    w_outT = nc.declare_dram_parameter("w_outT", [C, C], F32R, isOutput=False)
    affn = nc.declare_dram_parameter("affn", [4, C], F32, isOutput=False)
    y_out = nc.declare_dram_parameter("y", [C, N], F32, isOutput=True)
    debug = bool(int(os.environ.get("KERNEL_DEBUG", "0")))
    if debug:
        dbg_exp = nc.declare_dram_parameter(
            "dbg_exp", [P, NCH, J_L], F32, isOutput=True)


    # [p, ch, n] views of the channel-major DRAM tensors
    x_r = x_in[:, :].rearrange("(c p) n -> p c n", p=P)
    y_r = y_out[:, :].rearrange("(c p) n -> p c n", p=P)
    # [t, p, s, j] view of kv^T rows grouped per 512-col tile
    kvt_r = kvt_in[:, :].rearrange("(t s p) j -> t p s j", p=P, s=NSUB)
    # [t, p, s, n] view of kv with the 512 key rows split into 4 chunks
    kv_r = kv_in[:, :].rearrange("(s p) (t n) -> t p s n", p=P, n=TILE)
    affn_r = affn[:, :].rearrange("a (c p) -> p a c", p=P)

    with _TC(nc) as tc:
        pid = nc.partition_id()

        import contextlib
        stack = contextlib.ExitStack()
        with stack:
            singles = stack.enter_context(tc.tile_pool(name="singles", bufs=1))
            res = stack.enter_context(tc.tile_pool(name="res", bufs=1))

            # ---- persistent tiles -------------------------------------
            emb_res = res.tile([P, NCH, N], F16)
            z_res = res.tile([P, NCH, N], F16)

            w_inT_s = singles.tile([P, NCH, C], F16)
            wqT_s = singles.tile([P, NCH, C], F32R)
            wkT_s = singles.tile([P, NCH, C], F32R)
            wvT_s = singles.tile([P, NCH, C], F32R)
            woT_s = singles.tile([P, NCH, C], F32R)
            w_outT_s = singles.tile([P, NCH, C], F32R)
            for dst, src in ((w_inT_s, w_inT), (wqT_s, wqT), (wkT_s, wkT),
                             (wvT_s, wvT), (woT_s, woT), (w_outT_s, w_outT)):
                nc.sync.dma_start(
                    out=dst, in_=src[:, :].rearrange("(c p) o -> p c o", p=P))

            affn_s = singles.tile([P, 4, NCH], F32)
            nc.sync.dma_start(out=affn_s, in_=affn_r)

            ident_bf = singles.tile([P, P], BF16)
            make_identity(nc, ident_bf)
            ident_f = singles.tile([P, P], F32)
            make_identity(nc, ident_f)

            eps_t = singles.tile([P, 1], F32)
            nc.vector.memset(eps_t, EPS)

            st_e = singles.tile([P, NCH, nt, 6], F32)
            st_z = singles.tile([P, NCH, nt, 6], F32)
            st_z1_sum = singles.tile([P, nt], F32)
            st_z1_sq = singles.tile([P, nt], F32)
            s1_t = singles.tile([P, NCH], F32)
            b1_t = singles.tile([P, NCH], F32)
            s2_t = singles.tile([P, NCH], F32)
            b2_t = singles.tile([P, NCH], F32)

            # ================= Phase A: map_in + stats =================
            def _phase_a(suffix):
                with tc.tile_pool(name=f"phA{suffix}", bufs=3) as ioA, \
                     tc.tile_pool(name=f"psA{suffix}", bufs=3,
                                  space="PSUM") as psA:
                    for t in range(nt):
                        ts = bass.ts(t, TILE)
                        x_t = ioA.tile([P, NCH, TILE], F16, tag="x")
                        nc.sync.dma_start(out=x_t, in_=x_r[:, :, ts])
                        e_ps = psA.tile([P, NCH, TILE], F32, tag="eps")
                        for oc in range(NCH):
                            for cc in range(NCH):
                                nc.tensor.matmul(
                                    e_ps[:, oc, :],
                                    w_inT_s[:, cc, bass.ts(oc, P)],
                                    x_t[:, cc, :],
                                    start=(cc == 0), stop=(cc == NCH - 1))
                        nc.scalar.activation(
                            out=emb_res[:, :, ts], in_=e_ps, func=AF.Gelu)
                        for oc in range(NCH):
                            nc.vector.bn_stats(
                                out=st_e[:, oc, t, :], in_=emb_res[:, oc, ts])

            # stats -> s1 = gamma*rstd, b1 = beta - mean*s1
            def _fold_stats(st, s_t, b_t, g_idx, b_idx):
                mv = singles.tile([P, NCH, 2], F32, tag=f"mv{g_idx}")
                sd = singles.tile([P, NCH], F32, tag=f"sd{g_idx}")
                tmp = singles.tile([P, NCH], F32, tag=f"tmp{g_idx}")
                for ch in range(NCH):
                    nc.vector.bn_aggr(out=mv[:, ch, :], in_=st[:, ch, :, :])
                    nc.scalar.activation(
                        out=sd[:, ch:ch + 1], in_=mv[:, ch, 1:2],
                        func=AF.Sqrt, bias=eps_t)
                    nc.vector.reciprocal(
                        out=sd[:, ch:ch + 1], in_=sd[:, ch:ch + 1])
                    nc.vector.tensor_mul(
                        s_t[:, ch:ch + 1], sd[:, ch:ch + 1],
                        affn_s[:, g_idx, ch:ch + 1])
                    nc.vector.tensor_mul(
                        tmp[:, ch:ch + 1], mv[:, ch, 0:1], s_t[:, ch:ch + 1])
                    nc.vector.tensor_sub(
                        b_t[:, ch:ch + 1], affn_s[:, b_idx, ch:ch + 1],
                        tmp[:, ch:ch + 1])


            inv_n = 1.0 / float(nt * TILE)

            def _fold_stats_z(act_ch1):
                mv = singles.tile([P, NCH, 2], F32, tag="mvz")
                sd = singles.tile([P, NCH], F32, tag="sdz")
                tmp = singles.tile([P, NCH], F32, tag="tmpz")
                nc.vector.bn_aggr(out=mv[:, 0, :], in_=st_z[:, 0, :, :])
                if act_ch1:
                    nc.vector.tensor_reduce(
                        out=mv[:, 1, 0:1], in_=st_z1_sum, axis=AX.X,
                        op=ALU.add)
                    nc.scalar.mul(out=mv[:, 1, 0:1], in_=mv[:, 1, 0:1],
                                  mul=inv_n)
                    nc.vector.tensor_reduce(
                        out=mv[:, 1, 1:2], in_=st_z1_sq, axis=AX.X,
                        op=ALU.add)
                    nc.scalar.mul(out=mv[:, 1, 1:2], in_=mv[:, 1, 1:2],
                                  mul=inv_n)
                    nc.vector.tensor_mul(
                        tmp[:, 1:2], mv[:, 1, 0:1], mv[:, 1, 0:1])
                    nc.vector.tensor_sub(
                        mv[:, 1, 1:2], mv[:, 1, 1:2], tmp[:, 1:2])
                else:
                    nc.vector.bn_aggr(out=mv[:, 1, :], in_=st_z[:, 1, :, :])
                for ch in range(NCH):
                    nc.scalar.activation(
                        out=sd[:, ch:ch + 1], in_=mv[:, ch, 1:2],
                        func=AF.Sqrt, bias=eps_t)
                    nc.vector.reciprocal(
                        out=sd[:, ch:ch + 1], in_=sd[:, ch:ch + 1])
                    nc.vector.tensor_mul(
                        s2_t[:, ch:ch + 1], sd[:, ch:ch + 1],
                        affn_s[:, 2, ch:ch + 1])
                    nc.vector.tensor_mul(
                        tmp[:, ch:ch + 1], mv[:, ch, 0:1], s2_t[:, ch:ch + 1])
                    nc.vector.tensor_sub(
                        b2_t[:, ch:ch + 1], affn_s[:, 3, ch:ch + 1],
                        tmp[:, ch:ch + 1])

            def _phase_d(suffix):
                with tc.tile_pool(name=f"phD{suffix}", bufs=3) as ioD, \
                     tc.tile_pool(name=f"psD{suffix}", bufs=3,
                                  space="PSUM") as psD:
                    for t in range(nt):
                        ts = bass.ts(t, TILE)
                        n2_t = ioD.tile([P, NCH, TILE], F32R, tag="n2")
                        for ch in range(NCH):
                            eng = nc.gpsimd if ch == 0 else nc.vector
                            eng.tensor_scalar(
                                out=n2_t[:, ch, :], in0=z_res[:, ch, ts],
                                scalar1=s2_t[:, ch:ch + 1],
                                scalar2=b2_t[:, ch:ch + 1],
                                op0=ALU.mult, op1=ALU.add)
                        out_t = ioD.tile([P, NCH, TILE], F32, tag="out")
                        o_ps = psD.tile([P, NCH, TILE], F32, tag="ops")
                        for oc in range(NCH):
                            for cc in range(NCH):
                                nc.tensor.matmul(
                                    o_ps[:, oc, :],
                                    w_outT_s[:, cc, bass.ts(oc, P)],
                                    n2_t[:, cc, :],
                                    start=(cc == 0), stop=(cc == NCH - 1))
                        nc.scalar.activation(out=out_t, in_=o_ps, func=AF.Gelu)
                        nc.sync.dma_start(out=y_r[:, :, ts], in_=out_t)

            # ================= Branch: labeled / unlabeled =============
            def _norm_tile(pool, t):
                ts = bass.ts(t, TILE)
                norm_t = pool.tile([P, NCH, TILE], F32R, tag="norm")
                for ch in range(NCH):
                    nc.gpsimd.tensor_scalar(
                        out=norm_t[:, ch, :], in0=emb_res[:, ch, ts],
                        scalar1=s1_t[:, ch:ch + 1], scalar2=b1_t[:, ch:ch + 1],
                        op0=ALU.mult, op1=ALU.add)
                return norm_t

            def _softmax_transpose(scores_ps, j_dim, attn_dt, attnT_dt, ident,
                                   pool, pst):
                """scores [P, NCH, j] psum -> attnT [P, j//P, C] (attn_dt)."""
                attn_pre = pool.tile([P, NCH, j_dim], F32, tag="attnpre")
                attn_sm = pool.tile([P, NCH, j_dim], attn_dt, tag="attnsm")
                ms = pool.tile([P, NCH, 6], F32, tag="sm_stats")
                mv = pool.tile([P, NCH, 2], F32, tag="sm_mv")
                sd = pool.tile([P, NCH], F32, tag="sm_sd")
                mx = pool.tile([P, NCH], F32, tag="sm_mx")
                sm = pool.tile([P, NCH], F32, tag="sm_sum")
                for ch in range(NCH):
                    nc.vector.bn_stats(
                        out=ms[:, ch, :], in_=scores_ps[ch])
                    nc.vector.bn_aggr(out=mv[:, ch, :], in_=ms[:, ch, :])
                    nc.scalar.activation(
                        out=sd[:, ch:ch + 1], in_=mv[:, ch, 1:2],
                        func=AF.Sqrt, bias=eps_t)
                    nc.vector.reciprocal(
                        out=sd[:, ch:ch + 1], in_=sd[:, ch:ch + 1])
                    nc.vector.tensor_scalar(
                        out=attn_pre[:, ch, :], in0=scores_ps[ch],
                        scalar1=mv[:, ch, 0:1], scalar2=sd[:, ch:ch + 1],
                        op0=ALU.subtract, op1=ALU.mult)
                    nc.vector.tensor_reduce(
                        out=mx[:, ch:ch + 1], in_=attn_pre[:, ch, :],
                        axis=AX.X, op=ALU.max, negate=True)
                    nc.scalar.activation(
                        out=attn_pre[:, ch, :], in_=attn_pre[:, ch, :],
                        func=AF.Exp, bias=mx[:, ch:ch + 1],
                        accum_out=sm[:, ch:ch + 1])
                    nc.vector.reciprocal(
                        out=sm[:, ch:ch + 1], in_=sm[:, ch:ch + 1])
                    nc.vector.tensor_scalar_mul(
                        out=attn_sm[:, ch, :], in0=attn_pre[:, ch, :],
                        scalar1=sm[:, ch:ch + 1])
                if debug:
                    nc.sync.dma_start(
                        out=dbg_exp[:, :, 0:j_dim],
                        in_=attn_pre[:, :, :])
                n_j = j_dim // P
                attnT = pool.tile([P, n_j, C], attnT_dt, tag="attnT")
                for ch in range(NCH):
                    for s in range(n_j):
                        tp_ps = pst.tile([P, P], attn_dt, tag="tp")
                        nc.tensor.transpose(
                            tp_ps, attn_sm[:, ch, bass.ts(s, P)], ident)
                        nc.scalar.copy(
                            out=attnT[:, s, bass.ts(ch, P)], in_=tp_ps)
                return attnT

            def _z_tile(psZ, zpool, t, ca_t, act_stats=False):
                """z = woT.T@ca + emb; stats; SBUF-resident."""
                ts = bass.ts(t, TILE)
                for oc in range(NCH):
                    z_ps = psZ.tile([P, TILE], F32, tag="zps")
                    for ic in range(NCH):
                        nc.tensor.matmul(
                            z_ps, woT_s[:, ic, bass.ts(oc, P)], ca_t[:, ic, :],
                            start=(ic == 0), stop=(ic == NCH - 1))
                    nc.vector.tensor_add(
                        z_res[:, oc, ts], z_ps, emb_res[:, oc, ts])
                    if oc == 0:
                        nc.vector.bn_stats(
                            out=st_z[:, oc, t, :], in_=z_res[:, oc, ts])
                    elif act_stats:
                        sscr = zpool.tile([P, TILE], F16, tag="sscr")
                        nc.scalar.activation(
                            out=sscr, in_=z_res[:, oc, ts], func=AF.Identity,
                            accum_out=st_z1_sum[:, t:t + 1])
                        qscr = zpool.tile([P, TILE], F16, tag="qscr")
                        nc.scalar.activation(
                            out=qscr, in_=z_res[:, oc, ts], func=AF.Square,
                            accum_out=st_z1_sq[:, t:t + 1])
                    else:
                        nc.vector.bn_stats(
                            out=st_z[:, oc, t, :], in_=z_res[:, oc, ts])

            import contextlib as _ctxlib

            def _if_lab():
                return (tc.If(pid < 4) if branch is None
                        else _ctxlib.nullcontext())

            def _else(cmp):
                return (cmp.Else() if branch is None
                        else _ctxlib.nullcontext())

            lab_cm = _if_lab()
            with lab_cm as cmp:
              if branch in (None, "lab"):
                _phase_a("L")
                _fold_stats(st_e, s1_t, b1_t, 0, 1)
                # ======== LABELED: cross-attention vs kv queue =========
                with tc.tile_pool(name="smL", bufs=1) as smp:
                    with tc.tile_pool(name="phBL", bufs=BUFS_IO) as ioB, \
                         tc.tile_pool(name="psQL", bufs=2, space="PSUM") as psQ, \
                         tc.tile_pool(name="psSL", bufs=1, space="PSUM") as psS, \
                         tc.tile_pool(name="psTL", bufs=2, space="PSUM") as psT:
                        scores_ps = [
                            psS.tile([P, J_L], F32, tag=f"sc{ic}",
                                     name=f"scoresL{ic}")
                            for ic in range(NCH)]
                        for t in range(nt):
                            norm_t = _norm_tile(ioB, t)
                            kvt_t = ioB.tile([P, NSUB, J_L], BF16, tag="kvt")
                            nc.sync.dma_start(out=kvt_t, in_=kvt_r[t])
                            qt_t = ioB.tile([P, NSUB, C], BF16, tag="qt")
                            q_ps = psQ.tile([P, NSUB, C], F32, tag="qps")
                            for s in range(NSUB):
                                for cc in range(NCH):
                                    nc.tensor.matmul(
                                        q_ps[:, s, :],
                                        norm_t[:, cc, bass.ts(s, P)],
                                        wqT_s[:, cc, :],
                                        start=(cc == 0), stop=(cc == NCH - 1))
                            nc.scalar.mul(out=qt_t, in_=q_ps, mul=SCALE)
                            for s in range(NSUB):
                                for ic in range(NCH):
                                    nc.tensor.matmul(
                                        scores_ps[ic],
                                        qt_t[:, s, bass.ts(ic, P)],
                                        kvt_t[:, s, :],
                                        start=(t == 0 and s == 0),
                                        stop=(t == nt - 1 and s == NSUB - 1))
                        attnT = _softmax_transpose(
                            scores_ps, J_L, BF16, BF16, ident_bf, smp, psT)

                    with tc.tile_pool(name="phCL", bufs=BUFS_IO) as ioC, \
                         tc.tile_pool(name="psCL", bufs=2, space="PSUM") as psC, \
                         tc.tile_pool(name="psZL", bufs=2, space="PSUM") as psZ:
                        for t in range(nt):
                            kv_t = ioC.tile([P, NSUB, TILE], BF16, tag="kv")
                            nc.sync.dma_start(out=kv_t, in_=kv_r[t])
                            ca_t = ioC.tile([P, NCH, TILE], F32R, tag="ca")
                            ca_ps = psC.tile([P, NCH, TILE], F32, tag="caps")
                            for ic in range(NCH):
                                for s in range(NSUB):
                                    nc.tensor.matmul(
                                        ca_ps[:, ic, :],
                                        attnT[:, s, bass.ts(ic, P)],
                                        kv_t[:, s, :],
                                        start=(s == 0), stop=(s == NSUB - 1))
                            nc.scalar.copy(out=ca_t, in_=ca_ps)
                            _z_tile(psZ, ioC, t, ca_t, act_stats=True)
                        _fold_stats_z(act_ch1=True)
                    _phase_d("L")

            with _else(cmp):
              if branch in (None, "unl"):
                _phase_a("U")
                _fold_stats(st_e, s1_t, b1_t, 0, 1)
                # ======== UNLABELED: channel self-attention ============
                with tc.tile_pool(name="smU", bufs=1) as smp:
                    with tc.tile_pool(name="phBU", bufs=BUFS_IO) as ioB, \
                         tc.tile_pool(name="psQU", bufs=2, space="PSUM") as psQ, \
                         tc.tile_pool(name="psSU", bufs=1, space="PSUM") as psS, \
                         tc.tile_pool(name="psTU", bufs=2, space="PSUM") as psT:
                        scores_ps = [
                            psS.tile([P, J_U], F32, tag=f"sc{ic}",
                                     name=f"scoresU{ic}")
                            for ic in range(NCH)]
                        for t in range(nt):
                            norm_t = _norm_tile(ioB, t)
                            qt_t = ioB.tile([P, NSUB, C], F32R, tag="qt")
                            kt_t = ioB.tile([P, NSUB, C], F32R, tag="kt")
                            for sp in range(NSUB // 2):
                                q_ps = psQ.tile([P, 2, C], F32, tag="qps")
                                k_ps = psQ.tile([P, 2, C], F32, tag="kps")
                                for si in range(2):
                                    s = sp * 2 + si
                                    for cc in range(NCH):
                                        nc.tensor.matmul(
                                            q_ps[:, si, :],
                                            norm_t[:, cc, bass.ts(s, P)],
                                            wqT_s[:, cc, :],
                                            start=(cc == 0),
                                            stop=(cc == NCH - 1))
                                    for cc in range(NCH):
                                        nc.tensor.matmul(
                                            k_ps[:, si, :],
                                            norm_t[:, cc, bass.ts(s, P)],
                                            wkT_s[:, cc, :],
                                            start=(cc == 0),
                                            stop=(cc == NCH - 1))
                                nc.scalar.mul(
                                    out=qt_t[:, bass.ts(sp, 2), :],
                                    in_=q_ps, mul=SCALE)
                                nc.scalar.copy(
                                    out=kt_t[:, bass.ts(sp, 2), :], in_=k_ps)
                            for s in range(NSUB):
                                for ic in range(NCH):
                                    nc.tensor.matmul(
                                        scores_ps[ic],
                                        qt_t[:, s, bass.ts(ic, P)],
                                        kt_t[:, s, :],
                                        start=(t == 0 and s == 0),
                                        stop=(t == nt - 1 and s == NSUB - 1))
                        attnT = _softmax_transpose(
                            scores_ps, J_U, F32, F32R, ident_f, smp, psT)

                    with tc.tile_pool(name="phCU", bufs=BUFS_IO) as ioC, \
                         tc.tile_pool(name="psVU", bufs=2, space="PSUM") as psV, \
                         tc.tile_pool(name="psCU", bufs=2, space="PSUM") as psC, \
                         tc.tile_pool(name="psZU", bufs=2, space="PSUM") as psZ:
                        for t in range(nt):
                            norm_t = _norm_tile(ioC, t)
                            v_t = ioC.tile([P, NCH, TILE], F32R, tag="v")
                            for jc in range(NCH):
                                v_ps = psV.tile([P, TILE], F32, tag="vps")
                                for cc in range(NCH):
                                    nc.tensor.matmul(
                                        v_ps, wvT_s[:, cc, bass.ts(jc, P)],
                                        norm_t[:, cc, :],
                                        start=(cc == 0), stop=(cc == NCH - 1))
                                nc.scalar.copy(out=v_t[:, jc, :], in_=v_ps)
                            ca_t = ioC.tile([P, NCH, TILE], F32R, tag="ca")
                            for ic in range(NCH):
                                ca_ps = psC.tile([P, TILE], F32, tag="caps")
                                for jc in range(NCH):
                                    nc.tensor.matmul(
                                        ca_ps, attnT[:, jc, bass.ts(ic, P)],
                                        v_t[:, jc, :],
                                        start=(jc == 0), stop=(jc == NCH - 1))
                                nc.scalar.copy(out=ca_t[:, ic, :], in_=ca_ps)
                            _z_tile(psZ, ioC, t, ca_t)
                        _fold_stats_z(act_ch1=False)
                    _phase_d("U")


    _split_sync_waits(nc)
    return nc


_NC_CACHE = {}


def _get_nc(nt: int):
    if nt not in _NC_CACHE:
        _NC_CACHE[nt] = _build(nt)
    return _NC_CACHE[nt]


def make_in_maps(features, kv_queue, w_in, w_out, wq_sa, wk_sa, wv_sa, wo_sa,
                 wq_ca, wo_ca, attn_gamma, attn_beta, enc_gamma, enc_beta):
    features = np.ascontiguousarray(features, dtype=np.float32)
    twoB = features.shape[0]
    B = twoB // 2
    kv = np.ascontiguousarray(kv_queue, dtype=np.float32).reshape(-1, N)
    kv_bf = kv.astype(ml_dtypes.bfloat16)
    kvt_bf = np.ascontiguousarray(kv.T).astype(ml_dtypes.bfloat16)
    kv_zero = np.zeros_like(kv_bf)
    kvt_zero = np.zeros_like(kvt_bf)
    zero_w = np.zeros((C, C), np.float32)
    affn = np.ascontiguousarray(
        np.stack([attn_gamma, attn_beta, enc_gamma, enc_beta]), np.float32)

    def wT(w):
        return np.ascontiguousarray(np.asarray(w, np.float32).T)

    w_inT, w_outT = wT(w_in).astype(np.float16), wT(w_out)
    in_maps = []
    for b in range(twoB):
        lab = b < B
        in_maps.append({
            "x": np.ascontiguousarray(
                features[b].reshape(C, N)).astype(np.float16),
            "kvt": kvt_bf if lab else kvt_zero,
            "kv": kv_bf if lab else kv_zero,
            "w_inT": w_inT,
            "wqT": wT(wq_ca) if lab else wT(wq_sa),
            "wkT": zero_w if lab else wT(wk_sa),
            "wvT": zero_w if lab else wT(wv_sa),
            "woT": wT(wo_ca) if lab else wT(wo_sa),
            "w_outT": w_outT,
            "affn": affn,
        })
    return in_maps


def kernel(**inputs) -> np.ndarray:
    nt = int(os.environ.get("KERNEL_NT", NT_FULL))
    nc = _get_nc(nt)
    in_maps = make_in_maps(**inputs)
    last_err = None
    for _attempt in range(3):
        try:
            res = run_bass_kernel_spmd(nc, in_maps, core_ids=list(range(8)))
            break
        except Exception as e:  # axon workers flake transiently
            last_err = e
    else:
        raise last_err
    out = np.stack([res.results[b]["y"] for b in range(8)])
    return out.reshape(8, C, 128, 128).astype(np.float32)


if __name__ == "__main__":
    nt = int(os.environ.get("KERNEL_NT", NT_FULL))
    nc = _build(nt)
    n_inst = sum(len(bb.instructions) for fn in nc.m.functions
                 for bb in fn.blocks)
    print(f"built nt={nt}: {n_inst} instructions")
